# revision 3
# baseline (speedup 1.0000x reference)
"""Trainium2 Bass kernel for MeanGaussianExactFlow.

Math notes (derived from the nn.Module reference):
  - z_corrected == z exactly (the x_mean @ H.T terms cancel), so x_mean is
    never needed.
  - With S = lam*H@V@H.T = Q diag(e) Q^T (one host-side 64x64 symmetric
    eigendecomposition), the batched inverse inv(S + sigma_b^2 I) is
    Q diag(g_b) Q^T with g_b[m] = 1/(e_m + sigma_b^2).  So
    A_b = U G_b W with U = -0.5*V@H.T@Q [D,64], W = Q.T@H [64,D].
  - The bias chain b_b is tiny ([D] per batch) -> computed host-side in
    float64 along with W/U/g.
  - The only large compute is f_b = x_b @ A_b^T + b_b^T (8.6 GFLOP total),
    run on device as f_b^T = A_b @ x_b^T + b_b in bf16 with fp32 PSUM
    accumulation (rel err ~4e-3, well under the 2e-2 gate).

Device work per core (32 batches, pure data parallel over B):
  A^T prep per batch: one per-partition scale (w * g_b, alternating
  DVE/ACT), one K=64 matmul (lhsT=w*g_b, rhs=u^T) -> PSUM, one bf16 copy
  to SBUF (opposite engine).  Main compute per batch: 2 PE matmuls
  (lhsT = A_b^T stationary bf16 [128,128], rhs = x_b^T bf16 [128,512]
  moving) -> PSUM fp32 -> fused bias-add + bf16-cast copy (alternating
  DVE/ACT) -> bf16 store (alternating gpsimd-SWDGE / scalar-HWDGE).
  x^T arrives pre-transposed/pre-cast bf16 from host; f^T returns bf16
  and is transposed/upcast on host.  bf16 I/O halves HBM traffic in both
  directions vs the fp32 baseline, and the PE transposes are gone.
"""

import numpy as np

B, N, D, M = 256, 1024, 128, 64
NCORES = 8
BLOC = B // NCORES  # 32 batches per core
BB = 2  # batches per x-load DMA

_PROGRAM_CACHE = {}


def _build_program():
    if _PROGRAM_CACHE:
        return _PROGRAM_CACHE["nc"]
    import concourse.mybir as mybir
    import concourse.tile as tile
    from concourse import bacc
    from contextlib import ExitStack

    fp32 = mybir.dt.float32
    bf16 = mybir.dt.bfloat16
    nc = bacc.Bacc("TRN2", target_bir_lowering=False, debug=False)

    xt_d = nc.dram_tensor("xt", [BLOC, D, N], bf16, kind="ExternalInput")
    # wut: cols 0:D = w (Q^T H), cols D:2D = u^T; rows are the 64 eigen modes
    wut_d = nc.dram_tensor("wut", [M, 2 * D], bf16, kind="ExternalInput")
    # gb: cols 0:BLOC = bias [D rows], cols BLOC:2*BLOC = g^T [M rows]
    gb_d = nc.dram_tensor("gb", [D, 2 * BLOC], fp32, kind="ExternalInput")
    f_d = nc.dram_tensor("f", [BLOC, D, N], bf16, kind="ExternalOutput")

    with tile.TileContext(nc) as tc, ExitStack() as ctx:
        const = ctx.enter_context(tc.tile_pool(name="const", bufs=1))
        wut_s = const.tile([M, 2 * D], bf16)
        nc.sync.dma_start(wut_s[:], wut_d.ap())
        gb_s = const.tile([D, 2 * BLOC], fp32)
        nc.sync.dma_start(gb_s[:], gb_d.ap())
        w_s = wut_s[:, :D]
        ut_s = wut_s[:, D:]
        at_all = const.tile([D, BLOC * D], bf16)

        wg_pool = ctx.enter_context(tc.tile_pool(name="wg", bufs=3))
        xb_pool = ctx.enter_context(tc.tile_pool(name="xb", bufs=6))
        fb_pool = ctx.enter_context(tc.tile_pool(name="fb", bufs=8))
        ps_pool = ctx.enter_context(tc.tile_pool(name="ps", bufs=3, space="PSUM"))
        atp_pool = ctx.enter_context(tc.tile_pool(name="atp", bufs=2, space="PSUM"))

        def emit_at_prep(b):
            e_scale = nc.vector if b % 2 == 0 else nc.scalar
            e_copy = nc.scalar if b % 2 == 0 else nc.vector
            gcol = gb_s[:M, BLOC + b : BLOC + b + 1]
            wg = wg_pool.tile([M, D], bf16, tag="wg")
            if b % 2 == 0:
                e_scale.tensor_scalar_mul(wg[:], w_s, gcol)
            else:
                e_scale.mul(wg[:], w_s, gcol)
            atp = atp_pool.tile([D, D], fp32, tag="atp")
            nc.tensor.matmul(atp[:], wg[:], ut_s, start=True, stop=True)
            at_b = at_all[:, b * D : (b + 1) * D]
            if b % 2 == 0:
                e_copy.copy(at_b, atp[:])
            else:
                e_copy.tensor_copy(at_b, atp[:])
            return at_b

        at_sbs = {}
        for bp in range(0, BLOC, BB):
            xb = xb_pool.tile([D, BB, N], bf16, tag="xb")
            nc.sync.dma_start(
                xb[:], xt_d.ap()[bp : bp + BB].rearrange("c e n -> e c n")
            )
            for bi in range(BB):
                b = bp + bi
                if b not in at_sbs:
                    at_sbs[b] = emit_at_prep(b)
                if b + 2 < BLOC:
                    at_sbs[b + 2] = emit_at_prep(b + 2)
                at_b = at_sbs[b]
                ps = ps_pool.tile([D, 2, N // 2], fp32, tag="ps")
                for j in range(2):
                    nc.tensor.matmul(
                        ps[:, j, :],
                        at_b,
                        xb[:, bi, j * (N // 2) : (j + 1) * (N // 2)],
                        start=True,
                        stop=True,
                    )
                fb = fb_pool.tile([D, N], bf16, tag="fb")
                bcol = gb_s[:, b : b + 1]
                if b % 2 == 0:
                    nc.vector.tensor_scalar_add(
                        fb[:], ps[:].rearrange("e a n -> e (a n)"), bcol
                    )
                    nc.gpsimd.dma_start(f_d.ap()[b], fb[:])
                else:
                    nc.scalar.add(fb[:], ps[:].rearrange("e a n -> e (a n)"), bcol)
                    nc.scalar.dma_start(f_d.ap()[b], fb[:])

    nc.compile()
    _PROGRAM_CACHE["nc"] = nc
    return nc


def kernel(lam, x, H, sigma, z, V_prior, mu_prior):
    import jax
    import jax.numpy as jnp
    import ml_dtypes

    lam = float(np.asarray(lam))
    x = np.asarray(x, dtype=np.float32)
    H64 = np.asarray(H, dtype=np.float64)
    sigma64 = np.asarray(sigma, dtype=np.float64)
    z64 = np.asarray(z, dtype=np.float64)
    V64 = np.asarray(V_prior, dtype=np.float64)
    mu64 = np.asarray(mu_prior, dtype=np.float64)

    # ---- tiny shared prep in float64 (one 64x64 symmetric eigendecomp) ----
    I_D = np.eye(D)
    PHT = V64 @ H64.T                         # [D,M]
    S = lam * (H64 @ PHT)                     # [M,M] symmetric PSD
    S = 0.5 * (S + S.T)
    e, Q = np.linalg.eigh(S)
    U = -0.5 * (PHT @ Q)                      # [D,M]
    W = Q.T @ H64                             # [M,D]
    sig2 = sigma64**2
    g = 1.0 / (e[None, :] + sig2[:, None])    # [B,M]

    # bias chain (exact reference algebra, fp64)
    A = np.einsum("dm,bm,me->bde", U, g, W)   # [B,D,D]
    t1 = (PHT[None] / sig2[:, None, None]) @ z64[:, :, None]  # [B,D,1]
    tb1 = (I_D[None] + lam * A) @ t1
    tb2 = A @ mu64[None, :, None]
    bvec = (I_D[None] + 2.0 * lam * A) @ (tb1 + tb2)          # [B,D,1]
    bias = bvec[:, :, 0].astype(np.float32)                   # [B,D]

    bf = ml_dtypes.bfloat16
    wut = np.zeros((M, 2 * D), dtype=bf)
    wut[:, :D] = W.astype(np.float32).astype(bf)
    wut[:, D:] = U.T.astype(np.float32).astype(bf)

    # x^T per batch (one multithreaded jax-cpu pass for transpose+cast)
    cpu = jax.local_devices(backend="cpu")[0]
    with jax.default_device(cpu):
        to_bf = jax.jit(lambda a: jnp.transpose(a, (0, 2, 1)).astype(jnp.bfloat16))
        xt_all = np.asarray(to_bf(x))                         # [B,D,N] bf16

    nc = _build_program()

    in_maps = []
    for c in range(NCORES):
        lo, hi = c * BLOC, (c + 1) * BLOC
        gb = np.zeros((D, 2 * BLOC), dtype=np.float32)
        gb[:, :BLOC] = bias[lo:hi].T
        gb[:M, BLOC:] = g[lo:hi].T.astype(np.float32)
        in_maps.append(
            dict(xt=np.ascontiguousarray(xt_all[lo:hi]), wut=wut, gb=gb)
        )

    from concourse.bass_utils import run_bass_kernel_spmd

    res = run_bass_kernel_spmd(nc, in_maps, core_ids=list(range(NCORES)))
    ft = np.stack([np.asarray(r["f"]) for r in res.results])  # [8,BLOC,D,N]
    with jax.default_device(cpu):
        back = jax.jit(
            lambda a: jnp.transpose(a.reshape(B, D, N), (0, 2, 1)).astype(jnp.float32)
        )
        out = np.asarray(back(ft))
    return out


# revision 9
# speedup vs baseline: 1.0920x; 1.0920x over previous
"""Trainium2 Bass kernel for MeanGaussianExactFlow.

Math notes (derived from the nn.Module reference):
  - z_corrected == z exactly (the x_mean @ H.T terms cancel), so x_mean is
    never needed.
  - With S = lam*H@V@H.T = Q diag(e) Q^T (one host-side 64x64 symmetric
    eigendecomposition), the batched inverse inv(S + sigma_b^2 I) is
    Q diag(g_b) Q^T with g_b[m] = 1/(e_m + sigma_b^2).  So
    A_b = U G_b W with U = -0.5*V@H.T@Q [D,64], W = Q.T@H [64,D].
  - The bias chain b_b is tiny ([D] per batch) -> computed host-side in
    float64 along with U and the per-batch scaled factors wg_b = G_b W.
  - The only large compute is f_b = x_b @ A_b^T + b_b^T (8.6 GFLOP total),
    run on device as f_b^T = A_b @ x_b^T + b_b in bf16 with fp32 PSUM
    accumulation (rel err ~4e-3, well under the 2e-2 gate).

Device work per core (32 batches, pure data parallel over B):
  A^T prep per batch (spread through the loop with small lookahead): one
  K=64 matmul (lhsT=wg_b, rhs=u^T) -> PSUM -> bf16 copy to SBUF
  (alternating DVE/ACT).  Main compute per batch: 2 PE matmuls
  (lhsT = A_b^T stationary bf16 [128,128], rhs = x_b^T bf16 [128,512]
  moving) -> PSUM fp32 -> fused bias-add + bf16-cast copy (alternating
  DVE/ACT) -> bf16 store (alternating gpsimd-SWDGE / scalar-HWDGE).
  x^T arrives pre-transposed/pre-cast bf16 from host; f^T returns bf16
  and is transposed/upcast on host.  bf16 I/O halves HBM traffic in both
  directions vs the fp32 baseline, and the PE transposes are gone.  The
  last two batches use half-sized copies/stores to shorten the tail
  critical path.
"""

import numpy as np

B, N, D, M = 256, 1024, 128, 64
NCORES = 8
BLOC = B // NCORES  # 32 batches per core
BB = 2  # batches per x-load DMA
AT_LOOKAHEAD = 4

_PROGRAM_CACHE = {}


def _build_program():
    if _PROGRAM_CACHE:
        return _PROGRAM_CACHE["nc"]
    import concourse.mybir as mybir
    import concourse.tile as tile
    from concourse import bacc
    from contextlib import ExitStack

    fp32 = mybir.dt.float32
    bf16 = mybir.dt.bfloat16
    nc = bacc.Bacc("TRN2", target_bir_lowering=False, debug=False)

    xt_d = nc.dram_tensor("xt", [BLOC, D, N], bf16, kind="ExternalInput")
    # wu: cols 0:D = W (Q^T H), cols D:2D = u^T
    wu_d = nc.dram_tensor("wu", [M, 2 * D], bf16, kind="ExternalInput")
    g_d = nc.dram_tensor("g", [M, BLOC], fp32, kind="ExternalInput")
    bias_d = nc.dram_tensor("bias", [D, BLOC], fp32, kind="ExternalInput")
    f_d = nc.dram_tensor("f", [BLOC, D, N], bf16, kind="ExternalOutput")

    with tile.TileContext(nc) as tc, ExitStack() as ctx:
        const = ctx.enter_context(tc.tile_pool(name="const", bufs=1))
        # consts go via SWDGE: their descriptor gen runs on Pool, off the
        # HWDGE path that feeds the x loads (keeps the DMA track gap-free)
        wu_s = const.tile([M, 2 * D], bf16)
        nc.gpsimd.dma_start(wu_s[:], wu_d.ap())
        g_s = const.tile([M, BLOC], fp32)
        nc.gpsimd.dma_start(g_s[:], g_d.ap())
        bias_s = const.tile([D, BLOC], fp32)
        nc.gpsimd.dma_start(bias_s[:], bias_d.ap())
        w_s = wu_s[:, :D]
        ut_s = wu_s[:, D:]
        at_all = const.tile([D, BLOC * D], bf16)

        wg_pool = ctx.enter_context(tc.tile_pool(name="wg", bufs=4))
        xb_pool = ctx.enter_context(tc.tile_pool(name="xb", bufs=6))
        fb_pool = ctx.enter_context(tc.tile_pool(name="fb", bufs=8))
        ps_pool = ctx.enter_context(tc.tile_pool(name="ps", bufs=3, space="PSUM"))
        atp_pool = ctx.enter_context(tc.tile_pool(name="atp", bufs=2, space="PSUM"))

        def emit_at_prep(b):
            # per-batch scale on the (otherwise idle-ish) gpsimd engine
            wg = wg_pool.tile([M, D], bf16, tag="wg")
            nc.gpsimd.tensor_scalar_mul(wg[:], w_s, g_s[:, b : b + 1])
            atp = atp_pool.tile([D, D], fp32, tag="atp")
            nc.tensor.matmul(atp[:], wg[:], ut_s, start=True, stop=True)
            at_b = at_all[:, b * D : (b + 1) * D]
            if b % 2 == 0:
                nc.scalar.copy(at_b, atp[:])
            else:
                nc.vector.tensor_copy(at_b, atp[:])
            return at_b

        at_sbs = {}
        for b in range(AT_LOOKAHEAD):
            at_sbs[b] = emit_at_prep(b)

        for bp in range(0, BLOC, BB):
            xb = xb_pool.tile([D, BB, N], bf16, tag="xb")
            if bp == 0:
                for bi in range(BB):
                    nc.sync.dma_start(
                        xb[:, bi, :], xt_d.ap()[bp + bi]
                    )
            else:
                nc.sync.dma_start(
                    xb[:], xt_d.ap()[bp : bp + BB].rearrange("c e n -> e c n")
                )
            for bi in range(BB):
                b = bp + bi
                if b + AT_LOOKAHEAD < BLOC:
                    at_sbs[b + AT_LOOKAHEAD] = emit_at_prep(b + AT_LOOKAHEAD)
                at_b = at_sbs[b]
                ps = ps_pool.tile([D, 2, N // 2], fp32, tag="ps")
                for j in range(2):
                    nc.tensor.matmul(
                        ps[:, j, :],
                        at_b,
                        xb[:, bi, j * (N // 2) : (j + 1) * (N // 2)],
                        start=True,
                        stop=True,
                    )
                bcol = bias_s[:, b : b + 1]
                fb = fb_pool.tile([D, N], bf16, tag="fb")
                if b >= BLOC - 2:
                    # tail: half-sized copies+stores shorten the critical path
                    for j in range(2):
                        half = slice(j * (N // 2), (j + 1) * (N // 2))
                        if j == 0:
                            nc.vector.tensor_scalar_add(fb[:, half], ps[:, j, :], bcol)
                            nc.sync.dma_start(f_d.ap()[b][:, half], fb[:, half])
                        else:
                            nc.scalar.add(fb[:, half], ps[:, j, :], bcol)
                            nc.scalar.dma_start(f_d.ap()[b][:, half], fb[:, half])
                elif b % 2 == 0:
                    nc.vector.tensor_scalar_add(
                        fb[:], ps[:].rearrange("e a n -> e (a n)"), bcol
                    )
                    nc.gpsimd.dma_start(f_d.ap()[b], fb[:])
                else:
                    nc.scalar.add(fb[:], ps[:].rearrange("e a n -> e (a n)"), bcol)
                    nc.scalar.dma_start(f_d.ap()[b], fb[:])

    nc.compile()
    _PROGRAM_CACHE["nc"] = nc
    return nc


def kernel(lam, x, H, sigma, z, V_prior, mu_prior):
    import jax
    import jax.numpy as jnp
    import ml_dtypes

    lam = float(np.asarray(lam))
    x = np.asarray(x, dtype=np.float32)
    H64 = np.asarray(H, dtype=np.float64)
    sigma64 = np.asarray(sigma, dtype=np.float64)
    z64 = np.asarray(z, dtype=np.float64)
    V64 = np.asarray(V_prior, dtype=np.float64)
    mu64 = np.asarray(mu_prior, dtype=np.float64)

    # ---- tiny shared prep in float64 (one 64x64 symmetric eigendecomp) ----
    I_D = np.eye(D)
    PHT = V64 @ H64.T                         # [D,M]
    S = lam * (H64 @ PHT)                     # [M,M] symmetric PSD
    S = 0.5 * (S + S.T)
    e, Q = np.linalg.eigh(S)
    U = -0.5 * (PHT @ Q)                      # [D,M]
    W = Q.T @ H64                             # [M,D]
    sig2 = sigma64**2
    g = 1.0 / (e[None, :] + sig2[:, None])    # [B,M]

    # bias chain (exact reference algebra, fp64)
    A = np.einsum("dm,bm,me->bde", U, g, W)   # [B,D,D]
    t1 = (PHT[None] / sig2[:, None, None]) @ z64[:, :, None]  # [B,D,1]
    tb1 = (I_D[None] + lam * A) @ t1
    tb2 = A @ mu64[None, :, None]
    bvec = (I_D[None] + 2.0 * lam * A) @ (tb1 + tb2)          # [B,D,1]
    bias = bvec[:, :, 0].astype(np.float32)                   # [B,D]

    bf = ml_dtypes.bfloat16
    wu = np.empty((M, 2 * D), dtype=bf)
    wu[:, :D] = W.astype(np.float32).astype(bf)
    wu[:, D:] = U.T.astype(np.float32).astype(bf)

    # x^T per batch (one multithreaded jax-cpu pass for transpose+cast)
    cpu = jax.local_devices(backend="cpu")[0]
    with jax.default_device(cpu):
        to_bf = jax.jit(lambda a: jnp.transpose(a, (0, 2, 1)).astype(jnp.bfloat16))
        xt_all = np.asarray(to_bf(x))                         # [B,D,N] bf16

    nc = _build_program()

    in_maps = []
    for c in range(NCORES):
        lo, hi = c * BLOC, (c + 1) * BLOC
        in_maps.append(
            dict(
                xt=np.ascontiguousarray(xt_all[lo:hi]),
                wu=wu,
                g=np.ascontiguousarray(g[lo:hi].T.astype(np.float32)),
                bias=np.ascontiguousarray(bias[lo:hi].T),
            )
        )

    from concourse.bass_utils import run_bass_kernel_spmd

    res = run_bass_kernel_spmd(nc, in_maps, core_ids=list(range(NCORES)))
    ft = np.stack([np.asarray(r["f"]) for r in res.results])  # [8,BLOC,D,N]
    with jax.default_device(cpu):
        back = jax.jit(
            lambda a: jnp.transpose(a.reshape(B, D, N), (0, 2, 1)).astype(jnp.float32)
        )
        out = np.asarray(back(ft))
    return out
